# revision 1
# baseline (speedup 1.0000x reference)
"""CMoE hash-routed expert FFN on 8 NeuronCores (expert-parallel).

Host side (the shard/unshard steps): compute hash routing
e = (token_id % 5099) % 64, first-come slot assignment with capacity 512,
scatter tokens into a per-expert [E, D, C] buffer (transposed, bf16), and
shard 8 experts to each of the 8 cores along with that core's (transposed,
bf16) expert weights.  Device side: per expert
    h  = relu(A @ Wk^T)^2        [C, F]
    kv = h @ Wv^T                [C, D]
    r  = sigmoid(A @ Wr^T)       [C, D]
    out = r * kv
computed entirely in transposed form (contraction dim on SBUF partitions),
bf16 matmul operands with fp32 PSUM accumulation.  Host gathers each
token's slot back out of [E, D, C] and zeroes dropped tokens.
"""

import numpy as np
import ml_dtypes

import concourse.bass as bass
import concourse.mybir as mybir
import concourse.tile as tile
from concourse import bacc
from concourse.bass import ts
from concourse.bass_utils import run_bass_kernel_spmd

HASH_PRIME = 5099
B, T, D, F, E = 8, 4096, 512, 1792, 64
S = B * T
C = 512  # capacity = max(4, ceil(S/E))
N_CORES = 8
E_LOC = E // N_CORES  # experts per core

BF16 = mybir.dt.bfloat16
F32 = mybir.dt.float32

_NC = None  # cached compiled Bass program
LAST_RESULT = None  # BassKernelResults of the most recent run (for test.py)


def _build_nc(e_loc=E_LOC, d=D, f=F, c=C):
    """One SPMD program: each core computes e_loc experts' FFN."""
    kd = d // 128   # contraction tiles over D
    kf = f // 128   # contraction tiles over F
    nc = bacc.Bacc("TRN2", target_bir_lowering=False, debug=False,
                   num_devices=N_CORES)

    a_t = nc.dram_tensor("a_t", [e_loc, d, c], BF16, kind="ExternalInput")
    wk_t = nc.dram_tensor("wk_t", [e_loc, d, f], BF16, kind="ExternalInput")
    wr_t = nc.dram_tensor("wr_t", [e_loc, d, d], BF16, kind="ExternalInput")
    wv_t = nc.dram_tensor("wv_t", [e_loc, f, d], BF16, kind="ExternalInput")
    out_t = nc.dram_tensor("out_t", [e_loc, d, c], F32, kind="ExternalOutput")

    with tile.TileContext(nc) as tc:
        with (
            tc.tile_pool(name="wts", bufs=2) as wts,
            tc.tile_pool(name="acts", bufs=2) as acts,
            tc.tile_pool(name="ph", bufs=3, space="PSUM") as ph,
            tc.tile_pool(name="pr", bufs=3, space="PSUM") as pr,
            tc.tile_pool(name="pkv", bufs=2, space="PSUM") as pkv,
        ):
            # Three DMA rings: sync HWDGE (at, wv), scalar HWDGE (wk),
            # gpsimd SWDGE (wr + output stores).  A single ring (~190 GB/s)
            # can't stay ahead of the PE stream.  The r phase runs one
            # expert ahead of h/kv: r only needs at+wr (1MB), so it gives
            # the PE work while the big wk/wv transfers ramp up, and wv is
            # issued late so the sync FIFO delivers at(e+1) before wv(e).
            tiles = {}
            sigs = {}

            # Warm the PE (HAM throttles it to 1.2 GHz until ~3.4us of
            # sustained work) with matmuls on scratch data while the first
            # input DMAs ramp up; the result is never read.
            warm_l = wts.tile([128, 128], BF16, tag="warm_l")
            warm_r = wts.tile([128, c], BF16, tag="warm_r")
            nc.any.memset(warm_l[:], 0.0)
            nc.any.memset(warm_r[:], 0.0)
            # measured best at 16 (~247us); longer bridges (26/42) delayed
            # the first real matmul group more than they saved
            for _ in range(16):
                warm_p = pr.tile([128, c], F32, tag="psr")
                nc.tensor.matmul(warm_p[:], lhsT=warm_l[:], rhs=warm_r[:],
                                 start=True, stop=True)

            def loads_early(e):
                at = wts.tile([128, kd, c], BF16, tag="at")
                wk = wts.tile([128, kd, f], BF16, tag="wk")
                wr = wts.tile([128, kd, d], BF16, tag="wr")
                tiles[e] = (at, wk, wr)
                k_src = wk_t[e].rearrange("(ko p) f -> p ko f", p=128)
                nc.sync.dma_start(at[:], a_t[e].rearrange("(ko p) c -> p ko c", p=128))
                wr_src = wr_t[e].rearrange("(ko p) g -> p ko g", p=128)
                if e < 2:
                    # head: sync ring delivers at+wr fastest during the ramp
                    nc.sync.dma_start(wr[:], wr_src)
                else:
                    nc.gpsimd.dma_start(wr[:], wr_src)
                # f-halves: h ft-groups 0..kf/2-1 gate on half 0 only
                half = (kf // 2) * 128
                nc.scalar.dma_start(wk[:, :, :half], k_src[:, :, :half])
                nc.scalar.dma_start(wk[:, :, half:], k_src[:, :, half:])

            def emit_r(e):
                at, _, wr = tiles[e]
                sig = acts.tile([128, kd, c], F32, tag="sig")
                sigs[e] = sig
                for gt in range(kd):
                    psum_r = pr.tile([128, c], F32, tag="psr")
                    for kt in range(kd):
                        nc.tensor.matmul(
                            psum_r[:],
                            lhsT=wr[:, kt, ts(gt, 128)],
                            rhs=at[:, kt, :],
                            start=(kt == 0),
                            stop=(kt == kd - 1),
                        )
                    nc.scalar.activation(sig[:, gt, :], psum_r[:],
                                         mybir.ActivationFunctionType.Sigmoid)

            def emit_h_kv(e):
                at, wk, _ = tiles.pop(e)
                wv = wts.tile([128, kf, d], BF16, tag="wv")
                nc.sync.dma_start(wv[:], wv_t[e].rearrange("(fo p) g -> p fo g", p=128))

                # h^T[f, c] = (relu(Wk^T.T @ A^T))^2, bf16 for matmul 2
                hb = acts.tile([128, kf, c], BF16, tag="hb")
                for ft in range(kf):
                    psum_h = ph.tile([128, c], F32, tag="psh")
                    for kt in range(kd):
                        nc.tensor.matmul(
                            psum_h[:],
                            lhsT=wk[:, kt, ts(ft, 128)],
                            rhs=at[:, kt, :],
                            start=(kt == 0),
                            stop=(kt == kd - 1),
                        )
                    nc.scalar.activation(hb[:, ft, :], psum_h[:],
                                         mybir.ActivationFunctionType.Relu)
                    nc.vector.tensor_mul(hb[:, ft, :], hb[:, ft, :], hb[:, ft, :])

                # kv^T[dd, c] = Wv^T.T @ h^T ; out = sig * kv
                sig = sigs.pop(e)
                ob = acts.tile([128, kd, c], F32, tag="ob")
                for dt in range(kd):
                    psum_kv = pkv.tile([128, c], F32, tag="pskv")
                    for ft in range(kf):
                        nc.tensor.matmul(
                            psum_kv[:],
                            lhsT=wv[:, ft, ts(dt, 128)],
                            rhs=hb[:, ft, :],
                            start=(ft == 0),
                            stop=(ft == kf - 1),
                        )
                    nc.vector.tensor_mul(ob[:, dt, :], psum_kv[:], sig[:, dt, :])
                    # store each d-tile as it finishes; the last expert's
                    # stores ride the by-then-idle sync HWDGE ring (lower
                    # latency than SWDGE) to shorten the kernel tail
                    dst = out_t[e].rearrange("(ko p) c -> p ko c", p=128)[:, dt, :]
                    if e == e_loc - 1:
                        nc.sync.dma_start(dst, ob[:, dt, :])
                    else:
                        nc.gpsimd.dma_start(dst, ob[:, dt, :])

            loads_early(0)
            emit_r(0)
            if e_loc > 1:
                loads_early(1)
                emit_r(1)
            for e in range(e_loc):
                emit_h_kv(e)
                if e + 2 < e_loc:
                    loads_early(e + 2)
                    emit_r(e + 2)

    nc.compile()
    return nc


def _route(token_ids):
    tid = token_ids.reshape(S).astype(np.int64)
    e_idx = (tid % HASH_PRIME) % E
    order = np.argsort(e_idx, kind="stable")
    sorted_e = e_idx[order]
    starts = np.searchsorted(sorted_e, np.arange(E))
    pos = np.empty(S, np.int64)
    pos[order] = np.arange(S) - starts[sorted_e]
    kept = pos < C
    return e_idx, pos, kept


def kernel(x, token_ids, Wk, Wr, Wv):
    global _NC, LAST_RESULT
    if _NC is None:
        _NC = _build_nc()

    e_idx, pos, kept = _route(token_ids)

    bf16 = ml_dtypes.bfloat16
    xf = np.ascontiguousarray(x, dtype=np.float32).reshape(S, D)
    disp_t = np.zeros((E, D, C), np.float32)
    disp_t[e_idx[kept], :, pos[kept]] = xf[kept]
    a_t = disp_t.astype(bf16)

    wk_t = np.asarray(Wk, dtype=np.float32).transpose(0, 2, 1).astype(bf16)
    wr_t = np.asarray(Wr, dtype=np.float32).transpose(0, 2, 1).astype(bf16)
    wv_t = np.asarray(Wv, dtype=np.float32).transpose(0, 2, 1).astype(bf16)

    in_maps = [
        {
            "a_t": a_t[i * E_LOC:(i + 1) * E_LOC],
            "wk_t": wk_t[i * E_LOC:(i + 1) * E_LOC],
            "wr_t": wr_t[i * E_LOC:(i + 1) * E_LOC],
            "wv_t": wv_t[i * E_LOC:(i + 1) * E_LOC],
        }
        for i in range(N_CORES)
    ]

    LAST_RESULT = run_bass_kernel_spmd(_NC, in_maps, list(range(N_CORES)))
    out_t = np.concatenate(
        [LAST_RESULT.results[i]["out_t"] for i in range(N_CORES)], axis=0)

    yf = out_t[e_idx, :, np.minimum(pos, C - 1)]
    yf[~kept] = 0.0
    return np.ascontiguousarray(yf.reshape(B, T, D), dtype=np.float32)

